# revision 16
# baseline (speedup 1.0000x reference)
"""Trainium2 Bass kernel for nn_MultiHeadAttention (B=2, S=2048, D=1024, H=16).

Reference computation (fp32):
    qh/kh/vh = split_heads(x @ W.T + b)
    scores   = (qh @ kh.T) / 8, masked (mask==0 -> -1e9)
    attnw    = softmax(scores)                        # [B,H,S,S] output 1
    attn_out = concat_heads(attnw @ vh)
    out      = (attn_out @ Wd.T + bd) @ Wd.T + bd     # dense applied twice

Sharding: 8 cores = 2 batches x 4 head-groups (4 heads each). Every core
processes the full sequence for its (batch, head-group), which keeps the
mask block structure identical across cores (required: SPMD = one program).
The dense layers need all heads: attention outputs are AllGathered
(feature-major) within each 4-core batch group, each core computes its
256-feature slice of dense1, the y1 slices are AllGathered again, and each
core computes its 256-feature slice of dense2. Both collectives and the
dense work are issued per 512-column query chunk so they pipeline under
the attention of later chunks.

Layout: everything feature-major ("transposed") so each matmul is a
natural PE op (out = lhsT.T @ rhs) with no transposes anywhere:
  - host pre-transposes q/k/v and the (sliced) weights
  - scores are computed transposed [k, q]: probs land k-major, exactly
    what the attnw @ V contraction wants
  - softmax skips the row-max (scores are O(1), exp cannot overflow); the
    mask is applied multiplicatively after exp (masked entries exact 0,
    matching the reference's exp(-1e9 - max) == 0); row sums ride along as
    a ones-column appended to V (one extra PSUM row of the same matmuls)
  - attn_weights are written transposed; the host transposes them back
    during unsharding
Precision/perf: projections and dense layers run float32r (full PE rate at
K=M=128). The score and attnw@V matmuls have K=64 / M=65 where the fp32r
path drops to ~4 cycles/column, so those two run bf16 (HW-measured ~4x
faster); probs are kept in fp32 for the attn_weights output, with a bf16
copy feeding the V matmul. Mask-zeroed score blocks are skipped entirely
(zeros written with one strided DMA per run); fully-kept blocks skip the
mask multiply. The program is specialized to the mask's block pattern
(rebuilt + cached per pattern).
"""

import os
import sys
import concurrent.futures as _fut
from contextlib import ExitStack

import numpy as np

for _p in ("/opt/trn_rl_repo", "/opt/pypackages"):
    if os.path.isdir(_p) and _p not in sys.path:
        sys.path.append(_p)

import concourse.bass as bass
import concourse.mybir as mybir
import concourse.tile as tile
from concourse import bacc
from concourse import bass_utils

F32 = mybir.dt.float32
F32R = mybir.dt.float32r
BF16 = mybir.dt.bfloat16
AF = mybir.ActivationFunctionType

B, S, D, H = 2, 2048, 1024, 16
DEPTH = D // H          # 64
NCORES = 8
HL = 4                  # heads per core
FL = HL * DEPTH         # 256 features per core
QC = 512                # query chunk (matmul free dim)
NQC = S // QC           # 4
KBS = 128               # key block (transposed-score partition dim)
NKB = S // KBS          # 16
SCALE = 1.0 / float(np.sqrt(DEPTH))
MAX_MASK_TILES = 64

_PROG_CACHE: dict = {}


def _rt(x):  # DRAM [C*128, N] -> [128, C, N] view for DMA
    return x.ap().rearrange("(c p) n -> p c n", p=128)


def _runs(lst):
    """Group a sorted int list into (start, len) runs."""
    runs = []
    for x in lst:
        if runs and x == runs[-1][0] + runs[-1][1]:
            runs[-1][1] += 1
        else:
            runs.append([x, 1])
    return runs


def _build_program(block_class):
    """block_class[qc][kb] in {'k' keep-all, 'z' all-masked, 'b' boundary}."""
    nc = bacc.Bacc("TRN2", target_bir_lowering=False, debug=False)

    qT = nc.dram_tensor("qT", [D, S], F32R, kind="ExternalInput")
    kT = nc.dram_tensor("kT", [D, S], F32R, kind="ExternalInput")
    vT = nc.dram_tensor("vT", [D, S], F32R, kind="ExternalInput")
    wqT = nc.dram_tensor("wqT", [D, FL], F32R, kind="ExternalInput")
    wkT = nc.dram_tensor("wkT", [D, FL], F32R, kind="ExternalInput")
    wvT = nc.dram_tensor("wvT", [D, FL], F32R, kind="ExternalInput")
    wdT = nc.dram_tensor("wdT", [D, D], F32R, kind="ExternalInput")
    wdTs = nc.dram_tensor("wdTs", [D, FL], F32R, kind="ExternalInput")
    bq = nc.dram_tensor("bq", [128, 2], F32, kind="ExternalInput")
    bk = nc.dram_tensor("bk", [128, 2], F32, kind="ExternalInput")
    bv = nc.dram_tensor("bv", [1, FL], F32, kind="ExternalInput")
    bd = nc.dram_tensor("bd", [128, 8], F32, kind="ExternalInput")
    bds = nc.dram_tensor("bds", [128, 2], F32, kind="ExternalInput")

    nb = sum(1 for qcr in block_class for c in qcr if c == "b")
    assert nb <= MAX_MASK_TILES, f"{nb} boundary blocks > {MAX_MASK_TILES}"
    maskb = None
    if nb:
        maskb = nc.dram_tensor("maskb", [nb, KBS, QC], F32R, kind="ExternalInput")

    attnwT = nc.dram_tensor("attnwT", [HL, S, S], F32, kind="ExternalOutput")
    outTs = nc.dram_tensor("outTs", [FL, S], F32, kind="ExternalOutput")
    sums_out = nc.dram_tensor("sums_out", [HL, S], F32, kind="ExternalOutput")

    cc1_in = [nc.dram_tensor(f"cc1_in{qc}", [FL, QC], F32R) for qc in range(NQC)]
    cc1_out = [nc.dram_tensor(f"cc1_out{qc}", [D, QC], F32R) for qc in range(NQC)]
    GROUPS = [[0, 1, 2, 3], [4, 5, 6, 7]]

    bidx_map = {}
    bi = 0
    for qc in range(NQC):
        for kb in range(NKB):
            if block_class[qc][kb] == "b":
                bidx_map[(qc, kb)] = bi
                bi += 1

    with tile.TileContext(nc) as tc, ExitStack() as ctx:

        def mmr(out, lhsT, rhs, start, stop):
            nc.tensor.matmul(
                out, lhsT.bitcast(F32R), rhs.bitcast(F32R), start=start, stop=stop
            )

        # PSUM pools (8 banks)
        ps_proj = ctx.enter_context(tc.tile_pool(name="ps_proj", bufs=2, space="PSUM"))
        ps_s = ctx.enter_context(tc.tile_pool(name="ps_s", bufs=3, space="PSUM"))
        ps_o = ctx.enter_context(tc.tile_pool(name="ps_o", bufs=2, space="PSUM"))
        ps_b = ctx.enter_context(tc.tile_pool(name="ps_b", bufs=1, space="PSUM"))

        base = ctx.enter_context(tc.tile_pool(name="base", bufs=1))
        bq_sb = base.tile([128, 2], F32, tag="bq")
        bk_sb = base.tile([128, 2], F32, tag="bk")
        bd_sb = base.tile([128, 8], F32, tag="bd")
        bds_sb = base.tile([128, 2], F32, tag="bds")
        nc.sync.dma_start(out=bq_sb[:], in_=bq.ap())
        nc.sync.dma_start(out=bk_sb[:], in_=bk.ap())
        nc.sync.dma_start(out=bd_sb[:], in_=bd.ap())
        nc.sync.dma_start(out=bds_sb[:], in_=bds.ap())
        bv_b = base.tile([128, FL], F32, tag="bv")
        nc.gpsimd.dma_start(out=bv_b[:], in_=bv.ap().to_broadcast((128, FL)))
        ones1f = base.tile([1, 128], F32, tag="ones1f")
        nc.vector.memset(ones1f[:], 1.0)
        ones1 = base.tile([1, 128], F32R, tag="ones1")
        nc.vector.tensor_copy(ones1[:], ones1f[:])
        onesc_f = base.tile([128, HL, 1], F32, tag="onescf")
        nc.vector.memset(onesc_f[:], 1.0)
        zero_t = base.tile([128, QC], F32, tag="zero")
        nc.vector.memset(zero_t[:], 0.0)

        qkvp = ctx.enter_context(tc.tile_pool(name="qkv", bufs=1))
        qhT_t = [qkvp.tile([128, S], BF16, tag=f"qhT{i}", name=f"qhT{i}") for i in range(2)]
        khT_t = [qkvp.tile([128, S], BF16, tag=f"khT{i}", name=f"khT{i}") for i in range(2)]
        vh_sb = [
            qkvp.tile([128, HL, DEPTH + 1], BF16, tag=f"vh{s}", name=f"vh{s}")
            for s in range(NKB)
        ]

        attnp = ctx.enter_context(tc.tile_pool(name="attn", bufs=1))
        densep = ctx.enter_context(tc.tile_pool(name="dense", bufs=1))

        # ---- projections (feature-major; interleaved by seq chunk) --------
        with tc.tile_pool(name="projw", bufs=1) as projw:
            wq_sb = projw.tile([128, 8, FL], F32R, tag="wq", name="wq")
            wk_sb = projw.tile([128, 8, FL], F32R, tag="wk", name="wk")
            wv_sb = projw.tile([128, 8, FL], F32R, tag="wv", name="wv")
            nc.sync.dma_start(out=wq_sb[:], in_=_rt(wqT))
            nc.sync.dma_start(out=wk_sb[:], in_=_rt(wkT))
            nc.sync.dma_start(out=wv_sb[:], in_=_rt(wvT))
            bv_v = bv_b[:].rearrange("p (h d) -> p h d", h=HL)

            with tc.tile_pool(name="chunks", bufs=1) as chunks:
                for ns in range(8):
                    nsl = slice(ns * 256, (ns + 1) * 256)
                    for xT, w_sb, b_sb, dst in (
                        (qT, wq_sb, bq_sb, qhT_t),
                        (kT, wk_sb, bk_sb, khT_t),
                    ):
                        xc = chunks.tile([128, 8, 256], F32R, tag="xchunk", name="xc")
                        nc.sync.dma_start(out=xc[:], in_=_rt(xT)[:, :, nsl])
                        for i in range(2):
                            ps = ps_proj.tile([128, 256], F32, tag="pp", name="pp")
                            for kc in range(8):
                                mmr(ps[:], w_sb[:, kc, i * 128 : (i + 1) * 128],
                                    xc[:, kc, :], start=(kc == 0), stop=(kc == 7))
                            nc.scalar.activation(
                                out=dst[i][:, nsl], in_=ps[:], func=AF.Identity,
                                bias=b_sb[:, i : i + 1], scale=1.0,
                            )
                    vc = chunks.tile([128, 8, 256], F32R, tag="xchunk", name="vc")
                    nc.sync.dma_start(out=vc[:], in_=_rt(vT)[:, :, nsl])
                    for sl in range(2):
                        s = ns * 2 + sl
                        ps = ps_proj.tile([128, FL], F32, tag="pp", name="ppv")
                        for kc in range(8):
                            mmr(ps[:], vc[:, kc, sl * 128 : (sl + 1) * 128],
                                wv_sb[:, kc, :], start=(kc == 0), stop=(kc == 7))
                        nc.vector.tensor_add(
                            vh_sb[s][:, :, 0:DEPTH],
                            ps[:].rearrange("p (h d) -> p h d", h=HL),
                            bv_v,
                        )
                        nc.vector.tensor_copy(
                            vh_sb[s][:, :, DEPTH : DEPTH + 1], onesc_f[:]
                        )

        wd_sb = densep.tile([128, 8, D], F32R, tag="wd", name="wd", bufs=1)
        nc.sync.dma_start(out=wd_sb[:], in_=_rt(wdT))
        wds_sb = densep.tile([128, 8, FL], F32R, tag="wds", name="wds", bufs=1)
        nc.sync.dma_start(out=wds_sb[:], in_=_rt(wdTs))

        # ---- attention + pipelined collectives/dense, per query chunk -----
        qc_order = sorted(
            range(NQC),
            key=lambda q: -sum(1 for c in block_class[q] if c != "z"),
        )
        for qc in qc_order:
            qsl = slice(qc * QC, (qc + 1) * QC)
            kbs = [kb for kb in range(NKB) if block_class[qc][kb] != "z"]
            zbs = [kb for kb in range(NKB) if block_class[qc][kb] == "z"]
            mt = {}
            for kb in kbs:
                if block_class[qc][kb] == "b":
                    m = attnp.tile([128, QC], F32R, tag="mtile", name="mt", bufs=4)
                    nc.sync.dma_start(out=m[:], in_=maskb.ap()[bidx_map[(qc, kb)]])
                    mt[kb] = m

            for h in range(HL):
                pt, ro = h // 2, (h % 2) * DEPTH
                probs = {}
                for kb in kbs:
                    pss = ps_s.tile([128, QC], F32, tag="pss", name="pss")
                    nc.tensor.matmul(
                        pss[:],
                        khT_t[pt][ro : ro + DEPTH, kb * KBS : (kb + 1) * KBS],
                        qhT_t[pt][ro : ro + DEPTH, qsl],
                        start=True, stop=True,
                    )
                    pr = attnp.tile([128, QC], F32R, tag="probs", name="pr", bufs=12)
                    nc.scalar.activation(
                        out=pr[:], in_=pss[:], func=AF.Exp, scale=SCALE
                    )
                    prb = attnp.tile([128, QC], BF16, tag="prb", name="prb", bufs=8)
                    if kb in mt:
                        nc.vector.tensor_mul(prb[:], pr[:], mt[kb][:])
                    else:
                        nc.vector.tensor_copy(prb[:], pr[:])
                    nc.sync.dma_start(
                        out=attnwT.ap()[h, kb * KBS : (kb + 1) * KBS, qsl],
                        in_=pr[:].bitcast(F32),
                    )
                    probs[kb] = pr
                    probs[kb, "b"] = prb
                po = ps_o.tile([DEPTH + 1, QC], F32, tag="po", name="po")
                for j, kb in enumerate(kbs):
                    nc.tensor.matmul(
                        po[:], vh_sb[kb][:, h, :], probs[kb, "b"][:],
                        start=(j == 0), stop=(j == len(kbs) - 1),
                    )
                sums = attnp.tile([1, QC], F32R, tag="sums", name="sums", bufs=3)
                nc.scalar.copy(sums[:], po[DEPTH : DEPTH + 1, :])
                nc.sync.dma_start(
                    out=sums_out.ap()[h, qsl], in_=sums[:].bitcast(F32)
                )
                pb = ps_b.tile([128, QC], F32, tag="pb", name="pb")
                mmr(pb[:], ones1[:], sums[:], start=True, stop=True)
                recip = attnp.tile([DEPTH, QC], F32, tag="recip", name="recip", bufs=3)
                nc.vector.reciprocal(out=recip[:], in_=pb[0:DEPTH, :])
                ao = attnp.tile([DEPTH, QC], F32R, tag="ao", name="ao", bufs=3)
                nc.vector.tensor_mul(ao[:], po[0:DEPTH, :], recip[:])
                nc.sync.dma_start(
                    out=cc1_in[qc].ap()[h * DEPTH : (h + 1) * DEPTH, :], in_=ao[:]
                )
                for z0, zn in _runs(zbs):
                    dst = (
                        attnwT.ap()[h, z0 * KBS : (z0 + zn) * KBS, qsl]
                        .rearrange("(r p) q -> p r q", p=128)
                    )
                    z = zero_t[:]
                    src = bass.AP(
                        tensor=z.tensor, offset=z.offset,
                        ap=[list(z.ap[0]), [0, zn], list(z.ap[1])],
                    )
                    nc.sync.dma_start(out=dst, in_=src)

            # dense, feature-sliced, pipelined per qc
            nc.gpsimd.collective_compute(
                "AllGather", mybir.AluOpType.bypass, replica_groups=GROUPS,
                ins=[cc1_in[qc].ap()], outs=[cc1_out[qc].ap()],
            )
            ag1 = densep.tile([128, 8, QC], F32R, tag="ag1", name="ag1", bufs=1)
            nc.sync.dma_start(out=ag1[:], in_=_rt(cc1_out[qc]))
            y1 = densep.tile([128, 8, QC], F32R, tag="y1", name="y1", bufs=1)
            for m in range(8):
                ps = ps_proj.tile([128, QC], F32, tag="pp", name="ppd1")
                for kc in range(8):
                    mmr(ps[:], wd_sb[:, kc, m * 128 : (m + 1) * 128],
                        ag1[:, kc, :], start=(kc == 0), stop=(kc == 7))
                nc.scalar.activation(
                    out=y1[:, m, :], in_=ps[:], func=AF.Identity,
                    bias=bd_sb[:, m : m + 1], scale=1.0,
                )
            for i in range(2):
                ps = ps_proj.tile([128, QC], F32, tag="pp", name="ppd2")
                for kc in range(8):
                    mmr(ps[:], wds_sb[:, kc, i * 128 : (i + 1) * 128],
                        y1[:, kc, :], start=(kc == 0), stop=(kc == 7))
                y2 = densep.tile([128, QC], F32, tag="y2", name="y2", bufs=2)
                nc.scalar.activation(
                    out=y2[:], in_=ps[:], func=AF.Identity,
                    bias=bds_sb[:, i : i + 1], scale=1.0,
                )
                nc.sync.dma_start(
                    out=outTs.ap()[i * 128 : (i + 1) * 128, qsl], in_=y2[:]
                )

    nc.compile()
    return nc, nb


def _classify(keep):
    bc = []
    tiles = []
    for qc in range(NQC):
        row = []
        sub_q = keep[qc * QC : (qc + 1) * QC]
        for kb in range(NKB):
            sub = sub_q[:, kb * KBS : (kb + 1) * KBS]
            if sub.all():
                row.append("k")
            elif not sub.any():
                row.append("z")
            else:
                row.append("b")
                tiles.append(np.ascontiguousarray(sub.T.astype(np.float32)))
        bc.append(tuple(row))
    return tuple(bc), tiles


def kernel(v, k, q, mask, wq_w, wq_b, wk_w, wk_b, wv_w, wv_b, dense_w, dense_b):
    v, k, q = (np.asarray(x, np.float32) for x in (v, k, q))
    mask = np.asarray(mask)
    wq_w, wk_w, wv_w, dense_w = (
        np.asarray(x, np.float32) for x in (wq_w, wk_w, wv_w, dense_w)
    )
    wq_b, wk_b, wv_b, dense_b = (
        np.asarray(x, np.float32) for x in (wq_b, wk_b, wv_b, dense_b)
    )

    keep = np.broadcast_to(mask.reshape(mask.shape[-2], mask.shape[-1]), (S, S)) != 0
    bc, mtiles = _classify(keep)

    if bc not in _PROG_CACHE:
        _PROG_CACHE[bc] = _build_program(bc)
    nc, nb = _PROG_CACHE[bc]
    bidx = [
        (qc, kb)
        for qc in range(NQC)
        for kb in range(NKB)
        if bc[qc][kb] == "b"
    ]

    maskb_np = np.stack(mtiles, axis=0) if nb else None

    wdT_np = np.ascontiguousarray(dense_w.T)
    bd_np = np.ascontiguousarray(dense_b.reshape(8, 128).T)
    xT = {}
    for b in range(B):
        xT[b] = {
            "qT": np.ascontiguousarray(q[b].T),
            "kT": np.ascontiguousarray(k[b].T),
            "vT": np.ascontiguousarray(v[b].T),
        }

    in_maps = []
    for c in range(NCORES):
        b, hg = c // 4, c % 4
        fsl = slice(hg * FL, (hg + 1) * FL)
        m = {
            "qT": xT[b]["qT"],
            "kT": xT[b]["kT"],
            "vT": xT[b]["vT"],
            "wqT": np.ascontiguousarray(wq_w[fsl].T),
            "wkT": np.ascontiguousarray(wk_w[fsl].T),
            "wvT": np.ascontiguousarray(wv_w[fsl].T),
            "wdT": wdT_np,
            "wdTs": np.ascontiguousarray(dense_w[fsl].T),
            "bq": np.ascontiguousarray(wq_b[fsl].reshape(2, 128).T),
            "bk": np.ascontiguousarray(wk_b[fsl].reshape(2, 128).T),
            "bv": np.ascontiguousarray(wv_b[fsl].reshape(1, FL)),
            "bd": bd_np,
            "bds": np.ascontiguousarray(dense_b[fsl].reshape(2, 128).T),
        }
        if nb:
            m["maskb"] = maskb_np
        in_maps.append(m)

    res = bass_utils.run_bass_kernel_spmd(nc, in_maps, core_ids=list(range(NCORES)))

    out = np.empty((B, S, D), np.float32)
    attnw = np.empty((B, H, S, S), np.float32)
    for c in range(NCORES):
        b, hg = c // 4, c % 4
        out[b][:, hg * FL : (hg + 1) * FL] = res.results[c]["outTs"].T

    def _fill(args):
        c, hl = args
        b, hg = c // 4, c % 4
        recip = 1.0 / res.results[c]["sums_out"][hl]
        dst = attnw[b, hg * HL + hl]
        np.multiply(res.results[c]["attnwT"][hl].T, recip[:, None], out=dst)
        for i, (qc, kb) in enumerate(bidx):
            dst[qc * QC : (qc + 1) * QC, kb * KBS : (kb + 1) * KBS] *= mtiles[i].T

    with _fut.ThreadPoolExecutor(max_workers=16) as exe:
        list(exe.map(_fill, [(c, hl) for c in range(NCORES) for hl in range(HL)]))

    return out, attnw


# revision 17
# speedup vs baseline: 1.1507x; 1.1507x over previous
"""Trainium2 Bass kernel for nn_MultiHeadAttention (B=2, S=2048, D=1024, H=16).

Reference computation (fp32):
    qh/kh/vh = split_heads(x @ W.T + b)
    scores   = (qh @ kh.T) / 8, masked (mask==0 -> -1e9)
    attnw    = softmax(scores)                        # [B,H,S,S] output 1
    attn_out = concat_heads(attnw @ vh)
    out      = (attn_out @ Wd.T + bd) @ Wd.T + bd     # dense applied twice

Sharding: 8 cores = 2 batches x 4 head-groups (4 heads each). Every core
processes the full sequence for its (batch, head-group), which keeps the
mask block structure identical across cores (required: SPMD = one program).
The dense layers need all heads: attention outputs are AllGathered
(feature-major) within each 4-core batch group, each core computes its
256-feature slice of dense1, the y1 slices are AllGathered again, and each
core computes its 256-feature slice of dense2. Both collectives and the
dense work are issued per 512-column query chunk so they pipeline under
the attention of later chunks.

Layout: everything feature-major ("transposed") so each matmul is a
natural PE op (out = lhsT.T @ rhs) with no transposes anywhere:
  - host pre-transposes q/k/v and the (sliced) weights
  - scores are computed transposed [k, q]: probs land k-major, exactly
    what the attnw @ V contraction wants
  - softmax skips the row-max (scores are O(1), exp cannot overflow); the
    mask is applied multiplicatively after exp (masked entries exact 0,
    matching the reference's exp(-1e9 - max) == 0); row sums ride along as
    a ones-column appended to V (one extra PSUM row of the same matmuls)
  - attn_weights are written transposed; the host transposes them back
    during unsharding
Precision/perf: projections and dense layers run float32r (full PE rate at
K=M=128). The score and attnw@V matmuls have K=64 / M=65 where the fp32r
path drops to ~4 cycles/column, so those two run bf16 (HW-measured ~4x
faster); probs are kept in fp32 for the attn_weights output, with a bf16
copy feeding the V matmul. Mask-zeroed score blocks are skipped entirely
(zeros written with one strided DMA per run); fully-kept blocks skip the
mask multiply. The program is specialized to the mask's block pattern
(rebuilt + cached per pattern).
"""

import os
import sys
import concurrent.futures as _fut
from contextlib import ExitStack

import numpy as np

for _p in ("/opt/trn_rl_repo", "/opt/pypackages"):
    if os.path.isdir(_p) and _p not in sys.path:
        sys.path.append(_p)

import concourse.bass as bass
import concourse.mybir as mybir
import concourse.tile as tile
from concourse import bacc
from concourse import bass_utils

F32 = mybir.dt.float32
F32R = mybir.dt.float32r
BF16 = mybir.dt.bfloat16
AF = mybir.ActivationFunctionType

B, S, D, H = 2, 2048, 1024, 16
DEPTH = D // H          # 64
NCORES = 8
HL = 4                  # heads per core
FL = HL * DEPTH         # 256 features per core
QC = 512                # query chunk (matmul free dim)
NQC = S // QC           # 4
KBS = 128               # key block (transposed-score partition dim)
NKB = S // KBS          # 16
SCALE = 1.0 / float(np.sqrt(DEPTH))
MAX_MASK_TILES = 64

_PROG_CACHE: dict = {}


def _rt(x):  # DRAM [C*128, N] -> [128, C, N] view for DMA
    return x.ap().rearrange("(c p) n -> p c n", p=128)


def _runs(lst):
    """Group a sorted int list into (start, len) runs."""
    runs = []
    for x in lst:
        if runs and x == runs[-1][0] + runs[-1][1]:
            runs[-1][1] += 1
        else:
            runs.append([x, 1])
    return runs


def _build_program(block_class):
    """block_class[qc][kb] in {'k' keep-all, 'z' all-masked, 'b' boundary}."""
    nc = bacc.Bacc("TRN2", target_bir_lowering=False, debug=False)

    qT = nc.dram_tensor("qT", [D, S], F32R, kind="ExternalInput")
    kT = nc.dram_tensor("kT", [D, S], F32R, kind="ExternalInput")
    vT = nc.dram_tensor("vT", [D, S], F32R, kind="ExternalInput")
    wqT = nc.dram_tensor("wqT", [D, FL], F32R, kind="ExternalInput")
    wkT = nc.dram_tensor("wkT", [D, FL], F32R, kind="ExternalInput")
    wvT = nc.dram_tensor("wvT", [D, FL], F32R, kind="ExternalInput")
    wdT = nc.dram_tensor("wdT", [D, D], F32R, kind="ExternalInput")
    wdTs = nc.dram_tensor("wdTs", [D, FL], F32R, kind="ExternalInput")
    bq = nc.dram_tensor("bq", [128, 2], F32, kind="ExternalInput")
    bk = nc.dram_tensor("bk", [128, 2], F32, kind="ExternalInput")
    bv = nc.dram_tensor("bv", [1, FL], F32, kind="ExternalInput")
    bd = nc.dram_tensor("bd", [128, 8], F32, kind="ExternalInput")
    bds = nc.dram_tensor("bds", [128, 2], F32, kind="ExternalInput")

    nb = sum(1 for qcr in block_class for c in qcr if c == "b")
    assert nb <= MAX_MASK_TILES, f"{nb} boundary blocks > {MAX_MASK_TILES}"
    maskb = None
    if nb:
        maskb = nc.dram_tensor("maskb", [nb, KBS, QC], F32R, kind="ExternalInput")

    attnwT = nc.dram_tensor("attnwT", [HL, S, S], F32, kind="ExternalOutput")
    outTs = nc.dram_tensor("outTs", [FL, S], F32, kind="ExternalOutput")
    sums_out = nc.dram_tensor("sums_out", [HL, S], F32, kind="ExternalOutput")

    cc1_in = [nc.dram_tensor(f"cc1_in{qc}", [FL, QC], F32R) for qc in range(NQC)]
    cc1_out = [nc.dram_tensor(f"cc1_out{qc}", [D, QC], F32R) for qc in range(NQC)]
    GROUPS = [[0, 1, 2, 3], [4, 5, 6, 7]]

    bidx_map = {}
    bi = 0
    for qc in range(NQC):
        for kb in range(NKB):
            if block_class[qc][kb] == "b":
                bidx_map[(qc, kb)] = bi
                bi += 1

    with tile.TileContext(nc) as tc, ExitStack() as ctx:

        def mmr(out, lhsT, rhs, start, stop):
            nc.tensor.matmul(
                out, lhsT.bitcast(F32R), rhs.bitcast(F32R), start=start, stop=stop
            )

        # PSUM pools (8 banks)
        ps_proj = ctx.enter_context(tc.tile_pool(name="ps_proj", bufs=2, space="PSUM"))
        ps_s = ctx.enter_context(tc.tile_pool(name="ps_s", bufs=3, space="PSUM"))
        ps_o = ctx.enter_context(tc.tile_pool(name="ps_o", bufs=2, space="PSUM"))
        ps_b = ctx.enter_context(tc.tile_pool(name="ps_b", bufs=1, space="PSUM"))

        base = ctx.enter_context(tc.tile_pool(name="base", bufs=1))
        bq_sb = base.tile([128, 2], F32, tag="bq")
        bk_sb = base.tile([128, 2], F32, tag="bk")
        bd_sb = base.tile([128, 8], F32, tag="bd")
        bds_sb = base.tile([128, 2], F32, tag="bds")
        nc.sync.dma_start(out=bq_sb[:], in_=bq.ap())
        nc.sync.dma_start(out=bk_sb[:], in_=bk.ap())
        nc.sync.dma_start(out=bd_sb[:], in_=bd.ap())
        nc.sync.dma_start(out=bds_sb[:], in_=bds.ap())
        bv_b = base.tile([128, FL], F32, tag="bv")
        nc.gpsimd.dma_start(out=bv_b[:], in_=bv.ap().to_broadcast((128, FL)))
        ones1f = base.tile([1, 128], F32, tag="ones1f")
        nc.vector.memset(ones1f[:], 1.0)
        ones1 = base.tile([1, 128], F32R, tag="ones1")
        nc.vector.tensor_copy(ones1[:], ones1f[:])
        onesc_f = base.tile([128, HL, 1], F32, tag="onescf")
        nc.vector.memset(onesc_f[:], 1.0)
        zero_t = base.tile([128, QC], F32, tag="zero")
        nc.vector.memset(zero_t[:], 0.0)

        qkvp = ctx.enter_context(tc.tile_pool(name="qkv", bufs=1))
        qhT_t = [qkvp.tile([128, S], BF16, tag=f"qhT{i}", name=f"qhT{i}") for i in range(2)]
        khT_t = [qkvp.tile([128, S], BF16, tag=f"khT{i}", name=f"khT{i}") for i in range(2)]
        vh_sb = [
            qkvp.tile([128, HL, DEPTH + 1], BF16, tag=f"vh{s}", name=f"vh{s}")
            for s in range(NKB)
        ]

        attnp = ctx.enter_context(tc.tile_pool(name="attn", bufs=1))
        densep = ctx.enter_context(tc.tile_pool(name="dense", bufs=1))

        # ---- projections (feature-major; interleaved by seq chunk) --------
        with tc.tile_pool(name="projw", bufs=1) as projw:
            wq_sb = projw.tile([128, 8, FL], F32R, tag="wq", name="wq")
            wk_sb = projw.tile([128, 8, FL], F32R, tag="wk", name="wk")
            wv_sb = projw.tile([128, 8, FL], F32R, tag="wv", name="wv")
            nc.sync.dma_start(out=wq_sb[:], in_=_rt(wqT))
            nc.sync.dma_start(out=wk_sb[:], in_=_rt(wkT))
            nc.sync.dma_start(out=wv_sb[:], in_=_rt(wvT))
            bv_v = bv_b[:].rearrange("p (h d) -> p h d", h=HL)

            with tc.tile_pool(name="chunks", bufs=1) as chunks:
                for ns in range(8):
                    nsl = slice(ns * 256, (ns + 1) * 256)
                    for xT, w_sb, b_sb, dst in (
                        (qT, wq_sb, bq_sb, qhT_t),
                        (kT, wk_sb, bk_sb, khT_t),
                    ):
                        xc = chunks.tile([128, 8, 256], F32R, tag="xchunk", name="xc")
                        nc.sync.dma_start(out=xc[:], in_=_rt(xT)[:, :, nsl])
                        for i in range(2):
                            ps = ps_proj.tile([128, 256], F32, tag="pp", name="pp")
                            for kc in range(8):
                                mmr(ps[:], w_sb[:, kc, i * 128 : (i + 1) * 128],
                                    xc[:, kc, :], start=(kc == 0), stop=(kc == 7))
                            nc.scalar.activation(
                                out=dst[i][:, nsl], in_=ps[:], func=AF.Identity,
                                bias=b_sb[:, i : i + 1], scale=1.0,
                            )
                    vc = chunks.tile([128, 8, 256], F32R, tag="xchunk", name="vc")
                    nc.sync.dma_start(out=vc[:], in_=_rt(vT)[:, :, nsl])
                    for sl in range(2):
                        s = ns * 2 + sl
                        ps = ps_proj.tile([128, FL], F32, tag="pp", name="ppv")
                        for kc in range(8):
                            mmr(ps[:], vc[:, kc, sl * 128 : (sl + 1) * 128],
                                wv_sb[:, kc, :], start=(kc == 0), stop=(kc == 7))
                        nc.vector.tensor_add(
                            vh_sb[s][:, :, 0:DEPTH],
                            ps[:].rearrange("p (h d) -> p h d", h=HL),
                            bv_v,
                        )
                        nc.vector.tensor_copy(
                            vh_sb[s][:, :, DEPTH : DEPTH + 1], onesc_f[:]
                        )

        wd_sb = densep.tile([128, 8, D], F32R, tag="wd", name="wd", bufs=1)
        nc.sync.dma_start(out=wd_sb[:], in_=_rt(wdT))
        wds_sb = densep.tile([128, 8, FL], F32R, tag="wds", name="wds", bufs=1)
        nc.sync.dma_start(out=wds_sb[:], in_=_rt(wdTs))

        # ---- attention + pipelined collectives/dense, per query chunk -----
        for qc in range(NQC):
            qsl = slice(qc * QC, (qc + 1) * QC)
            kbs = [kb for kb in range(NKB) if block_class[qc][kb] != "z"]
            zbs = [kb for kb in range(NKB) if block_class[qc][kb] == "z"]
            mt = {}
            for kb in kbs:
                if block_class[qc][kb] == "b":
                    m = attnp.tile([128, QC], F32R, tag="mtile", name="mt", bufs=4)
                    nc.sync.dma_start(out=m[:], in_=maskb.ap()[bidx_map[(qc, kb)]])
                    mt[kb] = m

            for h in range(HL):
                pt, ro = h // 2, (h % 2) * DEPTH
                probs = {}
                for kb in kbs:
                    pss = ps_s.tile([128, QC], F32, tag="pss", name="pss")
                    nc.tensor.matmul(
                        pss[:],
                        khT_t[pt][ro : ro + DEPTH, kb * KBS : (kb + 1) * KBS],
                        qhT_t[pt][ro : ro + DEPTH, qsl],
                        start=True, stop=True,
                    )
                    pr = attnp.tile([128, QC], F32R, tag="probs", name="pr", bufs=12)
                    nc.scalar.activation(
                        out=pr[:], in_=pss[:], func=AF.Exp, scale=SCALE
                    )
                    prb = attnp.tile([128, QC], BF16, tag="prb", name="prb", bufs=8)
                    if kb in mt:
                        nc.vector.tensor_mul(prb[:], pr[:], mt[kb][:])
                    else:
                        nc.vector.tensor_copy(prb[:], pr[:])
                    nc.sync.dma_start(
                        out=attnwT.ap()[h, kb * KBS : (kb + 1) * KBS, qsl],
                        in_=pr[:].bitcast(F32),
                    )
                    probs[kb] = pr
                    probs[kb, "b"] = prb
                po = ps_o.tile([DEPTH + 1, QC], F32, tag="po", name="po")
                for j, kb in enumerate(kbs):
                    nc.tensor.matmul(
                        po[:], vh_sb[kb][:, h, :], probs[kb, "b"][:],
                        start=(j == 0), stop=(j == len(kbs) - 1),
                    )
                sums = attnp.tile([1, QC], F32R, tag="sums", name="sums", bufs=3)
                nc.scalar.copy(sums[:], po[DEPTH : DEPTH + 1, :])
                nc.sync.dma_start(
                    out=sums_out.ap()[h, qsl], in_=sums[:].bitcast(F32)
                )
                pb = ps_b.tile([128, QC], F32, tag="pb", name="pb")
                mmr(pb[:], ones1[:], sums[:], start=True, stop=True)
                recip = attnp.tile([DEPTH, QC], F32, tag="recip", name="recip", bufs=3)
                nc.vector.reciprocal(out=recip[:], in_=pb[0:DEPTH, :])
                ao = attnp.tile([DEPTH, QC], F32R, tag="ao", name="ao", bufs=3)
                nc.vector.tensor_mul(ao[:], po[0:DEPTH, :], recip[:])
                nc.sync.dma_start(
                    out=cc1_in[qc].ap()[h * DEPTH : (h + 1) * DEPTH, :], in_=ao[:]
                )
                for z0, zn in _runs(zbs):
                    dst = (
                        attnwT.ap()[h, z0 * KBS : (z0 + zn) * KBS, qsl]
                        .rearrange("(r p) q -> p r q", p=128)
                    )
                    z = zero_t[:]
                    src = bass.AP(
                        tensor=z.tensor, offset=z.offset,
                        ap=[list(z.ap[0]), [0, zn], list(z.ap[1])],
                    )
                    nc.sync.dma_start(out=dst, in_=src)

            # dense, feature-sliced, pipelined per qc
            nc.gpsimd.collective_compute(
                "AllGather", mybir.AluOpType.bypass, replica_groups=GROUPS,
                ins=[cc1_in[qc].ap()], outs=[cc1_out[qc].ap()],
            )
            ag1 = densep.tile([128, 8, QC], F32R, tag="ag1", name="ag1", bufs=1)
            nc.sync.dma_start(out=ag1[:], in_=_rt(cc1_out[qc]))
            y1 = densep.tile([128, 8, QC], F32R, tag="y1", name="y1", bufs=1)
            for m in range(8):
                ps = ps_proj.tile([128, QC], F32, tag="pp", name="ppd1")
                for kc in range(8):
                    mmr(ps[:], wd_sb[:, kc, m * 128 : (m + 1) * 128],
                        ag1[:, kc, :], start=(kc == 0), stop=(kc == 7))
                nc.scalar.activation(
                    out=y1[:, m, :], in_=ps[:], func=AF.Identity,
                    bias=bd_sb[:, m : m + 1], scale=1.0,
                )
            for i in range(2):
                ps = ps_proj.tile([128, QC], F32, tag="pp", name="ppd2")
                for kc in range(8):
                    mmr(ps[:], wds_sb[:, kc, i * 128 : (i + 1) * 128],
                        y1[:, kc, :], start=(kc == 0), stop=(kc == 7))
                y2 = densep.tile([128, QC], F32, tag="y2", name="y2", bufs=2)
                nc.scalar.activation(
                    out=y2[:], in_=ps[:], func=AF.Identity,
                    bias=bds_sb[:, i : i + 1], scale=1.0,
                )
                nc.sync.dma_start(
                    out=outTs.ap()[i * 128 : (i + 1) * 128, qsl], in_=y2[:]
                )

    nc.compile()
    return nc, nb


def _classify(keep):
    bc = []
    tiles = []
    for qc in range(NQC):
        row = []
        sub_q = keep[qc * QC : (qc + 1) * QC]
        for kb in range(NKB):
            sub = sub_q[:, kb * KBS : (kb + 1) * KBS]
            if sub.all():
                row.append("k")
            elif not sub.any():
                row.append("z")
            else:
                row.append("b")
                tiles.append(np.ascontiguousarray(sub.T.astype(np.float32)))
        bc.append(tuple(row))
    return tuple(bc), tiles


def kernel(v, k, q, mask, wq_w, wq_b, wk_w, wk_b, wv_w, wv_b, dense_w, dense_b):
    v, k, q = (np.asarray(x, np.float32) for x in (v, k, q))
    mask = np.asarray(mask)
    wq_w, wk_w, wv_w, dense_w = (
        np.asarray(x, np.float32) for x in (wq_w, wk_w, wv_w, dense_w)
    )
    wq_b, wk_b, wv_b, dense_b = (
        np.asarray(x, np.float32) for x in (wq_b, wk_b, wv_b, dense_b)
    )

    keep = np.broadcast_to(mask.reshape(mask.shape[-2], mask.shape[-1]), (S, S)) != 0
    bc, mtiles = _classify(keep)

    if bc not in _PROG_CACHE:
        _PROG_CACHE[bc] = _build_program(bc)
    nc, nb = _PROG_CACHE[bc]
    bidx = [
        (qc, kb)
        for qc in range(NQC)
        for kb in range(NKB)
        if bc[qc][kb] == "b"
    ]

    maskb_np = np.stack(mtiles, axis=0) if nb else None

    wdT_np = np.ascontiguousarray(dense_w.T)
    bd_np = np.ascontiguousarray(dense_b.reshape(8, 128).T)
    xT = {}
    for b in range(B):
        xT[b] = {
            "qT": np.ascontiguousarray(q[b].T),
            "kT": np.ascontiguousarray(k[b].T),
            "vT": np.ascontiguousarray(v[b].T),
        }

    in_maps = []
    for c in range(NCORES):
        b, hg = c // 4, c % 4
        fsl = slice(hg * FL, (hg + 1) * FL)
        m = {
            "qT": xT[b]["qT"],
            "kT": xT[b]["kT"],
            "vT": xT[b]["vT"],
            "wqT": np.ascontiguousarray(wq_w[fsl].T),
            "wkT": np.ascontiguousarray(wk_w[fsl].T),
            "wvT": np.ascontiguousarray(wv_w[fsl].T),
            "wdT": wdT_np,
            "wdTs": np.ascontiguousarray(dense_w[fsl].T),
            "bq": np.ascontiguousarray(wq_b[fsl].reshape(2, 128).T),
            "bk": np.ascontiguousarray(wk_b[fsl].reshape(2, 128).T),
            "bv": np.ascontiguousarray(wv_b[fsl].reshape(1, FL)),
            "bd": bd_np,
            "bds": np.ascontiguousarray(dense_b[fsl].reshape(2, 128).T),
        }
        if nb:
            m["maskb"] = maskb_np
        in_maps.append(m)

    res = bass_utils.run_bass_kernel_spmd(nc, in_maps, core_ids=list(range(NCORES)))

    out = np.empty((B, S, D), np.float32)
    attnw = np.empty((B, H, S, S), np.float32)
    for c in range(NCORES):
        b, hg = c // 4, c % 4
        out[b][:, hg * FL : (hg + 1) * FL] = res.results[c]["outTs"].T

    def _fill(args):
        c, hl = args
        b, hg = c // 4, c % 4
        recip = 1.0 / res.results[c]["sums_out"][hl]
        dst = attnw[b, hg * HL + hl]
        np.multiply(res.results[c]["attnwT"][hl].T, recip[:, None], out=dst)
        for i, (qc, kb) in enumerate(bidx):
            dst[qc * QC : (qc + 1) * QC, kb * KBS : (kb + 1) * KBS] *= mtiles[i].T

    with _fut.ThreadPoolExecutor(max_workers=16) as exe:
        list(exe.map(_fill, [(c, hl) for c in range(NCORES) for hl in range(HL)]))

    return out, attnw


# revision 18
# speedup vs baseline: 1.4484x; 1.2587x over previous
"""Trainium2 Bass kernel for nn_MultiHeadAttention (B=2, S=2048, D=1024, H=16).

Reference computation (fp32):
    qh/kh/vh = split_heads(x @ W.T + b)
    scores   = (qh @ kh.T) / 8, masked (mask==0 -> -1e9)
    attnw    = softmax(scores)                        # [B,H,S,S] output 1
    attn_out = concat_heads(attnw @ vh)
    out      = (attn_out @ Wd.T + bd) @ Wd.T + bd     # dense applied twice

Sharding: 8 cores = 2 batches x 4 head-groups (4 heads each). Every core
processes the full sequence for its (batch, head-group), which keeps the
mask block structure identical across cores (required: SPMD = one program).
The dense layers need all heads: attention outputs are AllGathered
(feature-major) within each 4-core batch group, each core computes its
256-feature slice of dense1, the y1 slices are AllGathered again, and each
core computes its 256-feature slice of dense2. Both collectives and the
dense work are issued per 512-column query chunk so they pipeline under
the attention of later chunks.

Layout: everything feature-major ("transposed") so each matmul is a
natural PE op (out = lhsT.T @ rhs) with no transposes anywhere:
  - host pre-transposes q/k/v and the (sliced) weights
  - scores are computed transposed [k, q]: probs land k-major, exactly
    what the attnw @ V contraction wants
  - softmax skips the row-max (scores are O(1), exp cannot overflow); the
    mask is applied multiplicatively after exp (masked entries exact 0,
    matching the reference's exp(-1e9 - max) == 0); row sums ride along as
    a ones-column appended to V (one extra PSUM row of the same matmuls)
  - attn_weights are written transposed; the host transposes them back
    during unsharding
Precision/perf: projections and dense layers run float32r (full PE rate at
K=M=128). The score and attnw@V matmuls have K=64 / M=65 where the fp32r
path drops to ~4 cycles/column, so those two run bf16 (HW-measured ~4x
faster); probs are kept in fp32 for the attn_weights output, with a bf16
copy feeding the V matmul. Mask-zeroed score blocks are skipped entirely
(zeros written with one strided DMA per run); fully-kept blocks skip the
mask multiply. The program is specialized to the mask's block pattern
(rebuilt + cached per pattern).
"""

import os
import sys
import concurrent.futures as _fut
from contextlib import ExitStack

import numpy as np

for _p in ("/opt/trn_rl_repo", "/opt/pypackages"):
    if os.path.isdir(_p) and _p not in sys.path:
        sys.path.append(_p)

import concourse.bass as bass
import concourse.mybir as mybir
import concourse.tile as tile
from concourse import bacc
from concourse import bass_utils

F32 = mybir.dt.float32
F32R = mybir.dt.float32r
BF16 = mybir.dt.bfloat16
AF = mybir.ActivationFunctionType

B, S, D, H = 2, 2048, 1024, 16
DEPTH = D // H          # 64
NCORES = 8
HL = 4                  # heads per core
FL = HL * DEPTH         # 256 features per core
QC = 512                # query chunk (matmul free dim)
NQC = S // QC           # 4
KBS = 128               # key block (transposed-score partition dim)
NKB = S // KBS          # 16
SCALE = 1.0 / float(np.sqrt(DEPTH))
MAX_MASK_TILES = 64

_PROG_CACHE: dict = {}


def _rt(x):  # DRAM [C*128, N] -> [128, C, N] view for DMA
    return x.ap().rearrange("(c p) n -> p c n", p=128)


def _runs(lst):
    """Group a sorted int list into (start, len) runs."""
    runs = []
    for x in lst:
        if runs and x == runs[-1][0] + runs[-1][1]:
            runs[-1][1] += 1
        else:
            runs.append([x, 1])
    return runs


def _build_program(block_class):
    """block_class[qc][kb] in {'k' keep-all, 'z' all-masked, 'b' boundary}."""
    nc = bacc.Bacc("TRN2", target_bir_lowering=False, debug=False)

    qT = nc.dram_tensor("qT", [D, S], F32R, kind="ExternalInput")
    kT = nc.dram_tensor("kT", [D, S], F32R, kind="ExternalInput")
    vT = nc.dram_tensor("vT", [D, S], F32R, kind="ExternalInput")
    wqT = nc.dram_tensor("wqT", [D, FL], F32R, kind="ExternalInput")
    wkT = nc.dram_tensor("wkT", [D, FL], F32R, kind="ExternalInput")
    wvT = nc.dram_tensor("wvT", [D, FL], F32R, kind="ExternalInput")
    wdT = nc.dram_tensor("wdT", [D, D], F32R, kind="ExternalInput")
    wdTs = nc.dram_tensor("wdTs", [D, FL], F32R, kind="ExternalInput")
    bq = nc.dram_tensor("bq", [128, 2], F32, kind="ExternalInput")
    bk = nc.dram_tensor("bk", [128, 2], F32, kind="ExternalInput")
    bv = nc.dram_tensor("bv", [1, FL], F32, kind="ExternalInput")
    bd = nc.dram_tensor("bd", [128, 8], F32, kind="ExternalInput")
    bds = nc.dram_tensor("bds", [128, 2], F32, kind="ExternalInput")

    nb = sum(1 for qcr in block_class for c in qcr if c == "b")
    assert nb <= MAX_MASK_TILES, f"{nb} boundary blocks > {MAX_MASK_TILES}"
    maskb = None
    if nb:
        maskb = nc.dram_tensor("maskb", [nb, KBS, QC], BF16, kind="ExternalInput")

    attnwT = nc.dram_tensor("attnwT", [HL, S, S], BF16, kind="ExternalOutput")
    outTs = nc.dram_tensor("outTs", [FL, S], F32, kind="ExternalOutput")
    sums_out = nc.dram_tensor("sums_out", [HL, S], F32, kind="ExternalOutput")

    cc1_in = [nc.dram_tensor(f"cc1_in{qc}", [FL, QC], F32R) for qc in range(NQC)]
    cc1_out = [nc.dram_tensor(f"cc1_out{qc}", [D, QC], F32R) for qc in range(NQC)]
    GROUPS = [[0, 1, 2, 3], [4, 5, 6, 7]]

    bidx_map = {}
    bi = 0
    for qc in range(NQC):
        for kb in range(NKB):
            if block_class[qc][kb] == "b":
                bidx_map[(qc, kb)] = bi
                bi += 1

    with tile.TileContext(nc) as tc, ExitStack() as ctx:

        def mmr(out, lhsT, rhs, start, stop):
            nc.tensor.matmul(
                out, lhsT.bitcast(F32R), rhs.bitcast(F32R), start=start, stop=stop
            )

        # PSUM pools (8 banks)
        ps_proj = ctx.enter_context(tc.tile_pool(name="ps_proj", bufs=2, space="PSUM"))
        ps_s = ctx.enter_context(tc.tile_pool(name="ps_s", bufs=3, space="PSUM"))
        ps_o = ctx.enter_context(tc.tile_pool(name="ps_o", bufs=2, space="PSUM"))
        ps_b = ctx.enter_context(tc.tile_pool(name="ps_b", bufs=1, space="PSUM"))

        base = ctx.enter_context(tc.tile_pool(name="base", bufs=1))
        bq_sb = base.tile([128, 2], F32, tag="bq")
        bk_sb = base.tile([128, 2], F32, tag="bk")
        bd_sb = base.tile([128, 8], F32, tag="bd")
        bds_sb = base.tile([128, 2], F32, tag="bds")
        nc.sync.dma_start(out=bq_sb[:], in_=bq.ap())
        nc.sync.dma_start(out=bk_sb[:], in_=bk.ap())
        nc.sync.dma_start(out=bd_sb[:], in_=bd.ap())
        nc.sync.dma_start(out=bds_sb[:], in_=bds.ap())
        bv_b = base.tile([128, FL], F32, tag="bv")
        nc.gpsimd.dma_start(out=bv_b[:], in_=bv.ap().to_broadcast((128, FL)))
        ones1f = base.tile([1, 128], F32, tag="ones1f")
        nc.vector.memset(ones1f[:], 1.0)
        ones1 = base.tile([1, 128], F32R, tag="ones1")
        nc.vector.tensor_copy(ones1[:], ones1f[:])
        onesc_f = base.tile([128, HL, 1], F32, tag="onescf")
        nc.vector.memset(onesc_f[:], 1.0)
        zero_t = base.tile([128, QC], BF16, tag="zero")
        nc.vector.memset(zero_t[:], 0.0)

        qkvp = ctx.enter_context(tc.tile_pool(name="qkv", bufs=1))
        qhT_t = [qkvp.tile([128, S], BF16, tag=f"qhT{i}", name=f"qhT{i}") for i in range(2)]
        khT_t = [qkvp.tile([128, S], BF16, tag=f"khT{i}", name=f"khT{i}") for i in range(2)]
        vh_sb = [
            qkvp.tile([128, HL, DEPTH + 1], BF16, tag=f"vh{s}", name=f"vh{s}")
            for s in range(NKB)
        ]

        attnp = ctx.enter_context(tc.tile_pool(name="attn", bufs=1))
        densep = ctx.enter_context(tc.tile_pool(name="dense", bufs=1))

        # ---- projections (feature-major; interleaved by seq chunk) --------
        with tc.tile_pool(name="projw", bufs=1) as projw:
            wq_sb = projw.tile([128, 8, FL], F32R, tag="wq", name="wq")
            wk_sb = projw.tile([128, 8, FL], F32R, tag="wk", name="wk")
            wv_sb = projw.tile([128, 8, FL], F32R, tag="wv", name="wv")
            nc.sync.dma_start(out=wq_sb[:], in_=_rt(wqT))
            nc.sync.dma_start(out=wk_sb[:], in_=_rt(wkT))
            nc.sync.dma_start(out=wv_sb[:], in_=_rt(wvT))
            bv_v = bv_b[:].rearrange("p (h d) -> p h d", h=HL)

            with tc.tile_pool(name="chunks", bufs=1) as chunks:
                for ns in range(4):
                    nsl = slice(ns * 512, (ns + 1) * 512)
                    for xT, w_sb, b_sb, dst in (
                        (qT, wq_sb, bq_sb, qhT_t),
                        (kT, wk_sb, bk_sb, khT_t),
                    ):
                        xc = chunks.tile([128, 8, 512], F32R, tag="xchunk", name="xc")
                        nc.sync.dma_start(out=xc[:], in_=_rt(xT)[:, :, nsl])
                        for i in range(2):
                            ps = ps_proj.tile([128, 512], F32, tag="pp", name="pp")
                            for kc in range(8):
                                mmr(ps[:], w_sb[:, kc, i * 128 : (i + 1) * 128],
                                    xc[:, kc, :], start=(kc == 0), stop=(kc == 7))
                            nc.scalar.activation(
                                out=dst[i][:, nsl], in_=ps[:], func=AF.Identity,
                                bias=b_sb[:, i : i + 1], scale=1.0,
                            )
                    vc = chunks.tile([128, 8, 512], F32R, tag="xchunk", name="vc")
                    nc.sync.dma_start(out=vc[:], in_=_rt(vT)[:, :, nsl])
                    for sl in range(4):
                        s = ns * 4 + sl
                        ps = ps_proj.tile([128, FL], F32, tag="pp", name="ppv")
                        for kc in range(8):
                            mmr(ps[:], vc[:, kc, sl * 128 : (sl + 1) * 128],
                                wv_sb[:, kc, :], start=(kc == 0), stop=(kc == 7))
                        nc.vector.tensor_add(
                            vh_sb[s][:, :, 0:DEPTH],
                            ps[:].rearrange("p (h d) -> p h d", h=HL),
                            bv_v,
                        )
                        nc.vector.tensor_copy(
                            vh_sb[s][:, :, DEPTH : DEPTH + 1], onesc_f[:]
                        )

        wd_sb = densep.tile([128, 8, D], F32R, tag="wd", name="wd", bufs=1)
        nc.sync.dma_start(out=wd_sb[:], in_=_rt(wdT))
        wds_sb = densep.tile([128, 8, FL], F32R, tag="wds", name="wds", bufs=1)
        nc.sync.dma_start(out=wds_sb[:], in_=_rt(wdTs))

        # ---- attention + pipelined collectives/dense, per query chunk -----
        for qc in range(NQC):
            qsl = slice(qc * QC, (qc + 1) * QC)
            kbs = [kb for kb in range(NKB) if block_class[qc][kb] != "z"]
            zbs = [kb for kb in range(NKB) if block_class[qc][kb] == "z"]
            mt = {}
            for kb in kbs:
                if block_class[qc][kb] == "b":
                    m = attnp.tile([128, QC], BF16, tag="mtile", name="mt", bufs=4)
                    nc.sync.dma_start(out=m[:], in_=maskb.ap()[bidx_map[(qc, kb)]])
                    mt[kb] = m

            for h in range(HL):
                pt, ro = h // 2, (h % 2) * DEPTH
                probs = {}
                for kb in kbs:
                    pss = ps_s.tile([128, QC], F32, tag="pss", name="pss")
                    nc.tensor.matmul(
                        pss[:],
                        khT_t[pt][ro : ro + DEPTH, kb * KBS : (kb + 1) * KBS],
                        qhT_t[pt][ro : ro + DEPTH, qsl],
                        start=True, stop=True,
                    )
                    prb = attnp.tile([128, QC], BF16, tag="prb", name="prb", bufs=14)
                    nc.scalar.activation(
                        out=prb[:], in_=pss[:], func=AF.Exp, scale=SCALE
                    )
                    if kb in mt:
                        nc.vector.tensor_mul(prb[:], prb[:], mt[kb][:])
                    nc.sync.dma_start(
                        out=attnwT.ap()[h, kb * KBS : (kb + 1) * KBS, qsl],
                        in_=prb[:],
                    )
                    probs[kb, "b"] = prb
                po = ps_o.tile([DEPTH + 1, QC], F32, tag="po", name="po")
                for j, kb in enumerate(kbs):
                    nc.tensor.matmul(
                        po[:], vh_sb[kb][:, h, :], probs[kb, "b"][:],
                        start=(j == 0), stop=(j == len(kbs) - 1),
                    )
                sums = attnp.tile([1, QC], F32R, tag="sums", name="sums", bufs=3)
                nc.scalar.copy(sums[:], po[DEPTH : DEPTH + 1, :])
                nc.sync.dma_start(
                    out=sums_out.ap()[h, qsl], in_=sums[:].bitcast(F32)
                )
                pb = ps_b.tile([128, QC], F32, tag="pb", name="pb")
                mmr(pb[:], ones1[:], sums[:], start=True, stop=True)
                recip = attnp.tile([DEPTH, QC], F32, tag="recip", name="recip", bufs=3)
                nc.vector.reciprocal(out=recip[:], in_=pb[0:DEPTH, :])
                ao = attnp.tile([DEPTH, QC], F32R, tag="ao", name="ao", bufs=3)
                nc.vector.tensor_mul(ao[:], po[0:DEPTH, :], recip[:])
                nc.sync.dma_start(
                    out=cc1_in[qc].ap()[h * DEPTH : (h + 1) * DEPTH, :], in_=ao[:]
                )
                for z0, zn in _runs(zbs):
                    dst = (
                        attnwT.ap()[h, z0 * KBS : (z0 + zn) * KBS, qsl]
                        .rearrange("(r p) q -> p r q", p=128)
                    )
                    z = zero_t[:]
                    src = bass.AP(
                        tensor=z.tensor, offset=z.offset,
                        ap=[list(z.ap[0]), [0, zn], list(z.ap[1])],
                    )
                    nc.sync.dma_start(out=dst, in_=src)

            # dense, feature-sliced, pipelined per qc
            nc.gpsimd.collective_compute(
                "AllGather", mybir.AluOpType.bypass, replica_groups=GROUPS,
                ins=[cc1_in[qc].ap()], outs=[cc1_out[qc].ap()],
            )
            ag1 = densep.tile([128, 8, QC], F32R, tag="ag1", name="ag1", bufs=1)
            nc.sync.dma_start(out=ag1[:], in_=_rt(cc1_out[qc]))
            y1 = densep.tile([128, 8, QC], F32R, tag="y1", name="y1", bufs=1)
            for m in range(8):
                ps = ps_proj.tile([128, QC], F32, tag="pp", name="ppd1")
                for kc in range(8):
                    mmr(ps[:], wd_sb[:, kc, m * 128 : (m + 1) * 128],
                        ag1[:, kc, :], start=(kc == 0), stop=(kc == 7))
                nc.scalar.activation(
                    out=y1[:, m, :], in_=ps[:], func=AF.Identity,
                    bias=bd_sb[:, m : m + 1], scale=1.0,
                )
            for i in range(2):
                ps = ps_proj.tile([128, QC], F32, tag="pp", name="ppd2")
                for kc in range(8):
                    mmr(ps[:], wds_sb[:, kc, i * 128 : (i + 1) * 128],
                        y1[:, kc, :], start=(kc == 0), stop=(kc == 7))
                y2 = densep.tile([128, QC], F32, tag="y2", name="y2", bufs=2)
                nc.scalar.activation(
                    out=y2[:], in_=ps[:], func=AF.Identity,
                    bias=bds_sb[:, i : i + 1], scale=1.0,
                )
                nc.sync.dma_start(
                    out=outTs.ap()[i * 128 : (i + 1) * 128, qsl], in_=y2[:]
                )

    nc.compile()
    return nc, nb


def _classify(keep):
    bc = []
    tiles = []
    for qc in range(NQC):
        row = []
        sub_q = keep[qc * QC : (qc + 1) * QC]
        for kb in range(NKB):
            sub = sub_q[:, kb * KBS : (kb + 1) * KBS]
            if sub.all():
                row.append("k")
            elif not sub.any():
                row.append("z")
            else:
                row.append("b")
                tiles.append(np.ascontiguousarray(sub.T.astype(np.float32)))
        bc.append(tuple(row))
    return tuple(bc), tiles


def kernel(v, k, q, mask, wq_w, wq_b, wk_w, wk_b, wv_w, wv_b, dense_w, dense_b):
    v, k, q = (np.asarray(x, np.float32) for x in (v, k, q))
    mask = np.asarray(mask)
    wq_w, wk_w, wv_w, dense_w = (
        np.asarray(x, np.float32) for x in (wq_w, wk_w, wv_w, dense_w)
    )
    wq_b, wk_b, wv_b, dense_b = (
        np.asarray(x, np.float32) for x in (wq_b, wk_b, wv_b, dense_b)
    )

    keep = np.broadcast_to(mask.reshape(mask.shape[-2], mask.shape[-1]), (S, S)) != 0
    bc, mtiles = _classify(keep)

    if bc not in _PROG_CACHE:
        _PROG_CACHE[bc] = _build_program(bc)
    nc, nb = _PROG_CACHE[bc]
    bidx = [
        (qc, kb)
        for qc in range(NQC)
        for kb in range(NKB)
        if bc[qc][kb] == "b"
    ]

    import ml_dtypes
    maskb_np = (
        np.stack(mtiles, axis=0).astype(ml_dtypes.bfloat16) if nb else None
    )

    wdT_np = np.ascontiguousarray(dense_w.T)
    bd_np = np.ascontiguousarray(dense_b.reshape(8, 128).T)
    xT = {}
    for b in range(B):
        xT[b] = {
            "qT": np.ascontiguousarray(q[b].T),
            "kT": np.ascontiguousarray(k[b].T),
            "vT": np.ascontiguousarray(v[b].T),
        }

    in_maps = []
    for c in range(NCORES):
        b, hg = c // 4, c % 4
        fsl = slice(hg * FL, (hg + 1) * FL)
        m = {
            "qT": xT[b]["qT"],
            "kT": xT[b]["kT"],
            "vT": xT[b]["vT"],
            "wqT": np.ascontiguousarray(wq_w[fsl].T),
            "wkT": np.ascontiguousarray(wk_w[fsl].T),
            "wvT": np.ascontiguousarray(wv_w[fsl].T),
            "wdT": wdT_np,
            "wdTs": np.ascontiguousarray(dense_w[fsl].T),
            "bq": np.ascontiguousarray(wq_b[fsl].reshape(2, 128).T),
            "bk": np.ascontiguousarray(wk_b[fsl].reshape(2, 128).T),
            "bv": np.ascontiguousarray(wv_b[fsl].reshape(1, FL)),
            "bd": bd_np,
            "bds": np.ascontiguousarray(dense_b[fsl].reshape(2, 128).T),
        }
        if nb:
            m["maskb"] = maskb_np
        in_maps.append(m)

    res = bass_utils.run_bass_kernel_spmd(nc, in_maps, core_ids=list(range(NCORES)))

    out = np.empty((B, S, D), np.float32)
    attnw = np.empty((B, H, S, S), np.float32)
    for c in range(NCORES):
        b, hg = c // 4, c % 4
        out[b][:, hg * FL : (hg + 1) * FL] = res.results[c]["outTs"].T

    def _fill(args):
        c, hl = args
        b, hg = c // 4, c % 4
        recip = 1.0 / res.results[c]["sums_out"][hl]
        dst = attnw[b, hg * HL + hl]
        np.multiply(
            res.results[c]["attnwT"][hl].T.astype(np.float32),
            recip[:, None],
            out=dst,
        )
        for i, (qc, kb) in enumerate(bidx):
            dst[qc * QC : (qc + 1) * QC, kb * KBS : (kb + 1) * KBS] *= mtiles[i].T

    with _fut.ThreadPoolExecutor(max_workers=16) as exe:
        list(exe.map(_fill, [(c, hl) for c in range(NCORES) for hl in range(HL)]))

    return out, attnw
